# revision 1
# baseline (speedup 1.0000x reference)
"""Mesa-layer memory kernel for Trainium2 (8 NeuronCores, data-parallel over B).

Math: the reference's T-step Sherman-Morrison / discounted-accumulation
recurrence has a closed form,
    R_final = (I + K^T K)^{-1}            (eps term is O(1e-6) relative)
    S_final^T = K^T diag(c) V,   c_t = prod_{s>t} gamma_s
so per memory b the output is
    out_b = Q_b @ (R_b @ S_b^T).
R is computed with Newton-Schulz iterations in residual form
    X <- X + X^T (I - A X)
(bf16 iterations + fp32 refinements; the residual form keeps the bf16
asymmetry of X out of the error floor). The T-contracted matmuls and the
query readout run in bf16 (validated ~3.3e-3 max-rel vs fp32 reference);
fp32->bf16 casts are spread across the otherwise-idle Scalar and GpSimd
engines.

Layout trick: timestep t maps to (partition p, slot r) via t = 16 p + r.
The contraction over t only requires K/V (and Q/out for the readout side)
to agree on the partition assignment, and this one makes every DMA a fully
contiguous 1 MB transfer (8 KB per partition), which the DMA engines run
near line rate, instead of 512 B strided pieces.

The suffix cumprod of gammas runs in log space: 16-step free-dim scans per
partition + one triangular matmul on the TensorEngine for the
cross-partition prefix (a 2048-step serial scan would cost ~10 us).

The 8 memories run as two pipelined groups of 4: group 0's Newton-Schulz
iterations are emitted interleaved with group 1's loads, and group 1's
iterations interleaved with group 0's readout, so the TensorEngine's FIFO
always has independent work behind each iteration's serial dependency.

Each core owns B/8 = 8 independent memories; no cross-core communication.
"""

import numpy as np

B, T, DK, DV, NQ = 64, 2048, 128, 128, 2048
NCORES = 8
BPC = B // NCORES          # memories per core
P = 128                    # partitions
R16 = T // P               # 16 row-slots per partition
GCLAMP = 1e-30             # gamma clamp before log (exact-0 gammas)

NS_BF = 5                  # Newton-Schulz iterations in bf16
NS_FP = 2                  # fp32 refinement iterations


def build_nc(ns_bf=NS_BF, ns_fp=NS_FP):
    import concourse.mybir as mybir
    import concourse.tile as tile
    from concourse import bacc
    from concourse.masks import make_identity, make_upper_triangular

    fp32 = mybir.dt.float32
    bf16 = mybir.dt.bfloat16
    AF = mybir.ActivationFunctionType
    OP = mybir.AluOpType
    AX = mybir.AxisListType
    NIT = ns_bf + ns_fp

    # Bacc (not raw Bass): its compile() pass splits multi-sem sync waits to
    # the 1-wait-per-instruction limit the TRN2 encodings require.
    nc = bacc.Bacc(trn_type="TRN2", target_bir_lowering=False, debug=False)
    keys = nc.dram_tensor("keys", [BPC, T, DK], fp32, kind="ExternalInput").ap()
    values = nc.dram_tensor("values", [BPC, T, DV], fp32, kind="ExternalInput").ap()
    gammas = nc.dram_tensor("gammas", [BPC, T], fp32, kind="ExternalInput").ap()
    queries = nc.dram_tensor("queries", [BPC, NQ, DK], fp32, kind="ExternalInput").ap()
    out = nc.dram_tensor("out", [BPC, NQ, DV], fp32, kind="ExternalOutput").ap()

    with tile.TileContext(nc) as tc:
        const = tc.alloc_tile_pool(name="const", bufs=1)
        gam = tc.alloc_tile_pool(name="gam", bufs=1)
        kp = tc.alloc_tile_pool(name="kp", bufs=3)
        vp = tc.alloc_tile_pool(name="vp", bufs=3)
        kvbp = tc.alloc_tile_pool(name="kvbp", bufs=2)
        qp = tc.alloc_tile_pool(name="qp", bufs=3)
        qbp = tc.alloc_tile_pool(name="qbp", bufs=1)
        qtp = tc.alloc_tile_pool(name="qtp", bufs=2)
        small = tc.alloc_tile_pool(name="small", bufs=1)
        xs = tc.alloc_tile_pool(name="xs", bufs=2)
        outp = tc.alloc_tile_pool(name="outp", bufs=2)
        ps_as = tc.alloc_tile_pool(name="ps_as", bufs=2, space="PSUM")
        ps_w = tc.alloc_tile_pool(name="ps_w", bufs=5, space="PSUM")

        ident = const.tile([P, P], fp32)
        make_identity(nc, ident)
        ident_bf = const.tile([P, P], bf16)
        make_identity(nc, ident_bf)
        # 4 identity blocks side by side, for group-batched I - A@X residuals
        ident4 = const.tile([P, 4 * P], fp32)
        for i in range(4):
            make_identity(nc, ident4[:, i * P : (i + 1) * P])
        # strict upper triangular (ones above diagonal) and all-ones, for the
        # cross-partition prefix-sum of per-partition gamma-log totals
        utri = const.tile([P, P], fp32)
        make_upper_triangular(nc, utri, val=1.0, diag=False)
        ones2 = const.tile([P, P], fp32)
        nc.gpsimd.memset(ones2[:], 1.0)

        # ---- phase 0: suffix cumprod of gammas (log space) ----
        # g16[p, i, r] = gamma[i, 16p + r]
        g16 = gam.tile([P, BPC, R16], fp32)
        nc.sync.dma_start(
            g16[:], gammas.rearrange("i (p r) -> p i r", r=R16)
        )
        g16f = g16.rearrange("p i r -> p (i r)")
        nc.vector.tensor_scalar_max(g16f, g16f, GCLAMP)
        nc.scalar.activation(g16f, g16f, AF.Ln)
        incl = gam.tile([P, BPC, R16], fp32)
        zz = gam.tile([P, R16], fp32)
        nc.vector.memset(zz[:], 0.0)
        # joiner: make DVE observe the ACT (Ln) dependency before the scans
        joiner = gam.tile([P, 1], fp32)
        nc.vector.tensor_copy(out=joiner[:], in_=g16[:, 0, 0:1])
        for i in range(BPC):
            nc.vector.tensor_tensor_scan(
                incl[:, i, :], g16[:, i, :], zz[:], 0.0, OP.add, OP.add
            )
        # per-partition totals -> cross-partition exclusive prefix + full sum
        ptot = gam.tile([P, BPC], fp32)
        nc.vector.tensor_copy(out=ptot[:], in_=incl[:, :, R16 - 1])
        ps_pre = ps_w.tile([P, 2 * BPC], fp32, tag="w", name="ps_pre")
        nc.tensor.matmul(ps_pre[:, 0:BPC], utri[:], ptot[:])     # offs
        nc.tensor.matmul(ps_pre[:, BPC : 2 * BPC], ones2[:], ptot[:])  # total
        pre_sb = gam.tile([P, 2 * BPC], fp32)
        nc.vector.tensor_copy(out=pre_sb[:], in_=ps_pre[:])
        bias2 = gam.tile([P, BPC], fp32)
        # bias = total - offs  (per partition & memory)
        nc.vector.tensor_tensor(
            bias2[:], pre_sb[:, BPC : 2 * BPC], pre_sb[:, 0:BPC], OP.subtract
        )
        # c_t[p, i, r] = exp(bias - incl) = prod_{s > 16p+r} gamma[i, s]
        c_t = gam.tile([P, BPC, R16], fp32)
        for i in range(BPC):
            nc.scalar.activation(
                c_t[:, i, :], incl[:, i, :], AF.Exp,
                bias=bias2[:, i : i + 1], scale=-1.0,
            )

        # ---- per-memory state tiles ----
        A_sb = [small.tile([P, P], fp32, tag=f"A{i}", name=f"A{i}") for i in range(BPC)]
        A_bf = [small.tile([P, P], bf16, tag=f"Ab{i}", name=f"Ab{i}") for i in range(BPC)]
        ST_sb = [small.tile([P, P], fp32, tag=f"S{i}", name=f"S{i}") for i in range(BPC)]
        Phi_bf = [small.tile([P, P], bf16, tag=f"Pb{i}", name=f"Phib{i}") for i in range(BPC)]
        rs_sb = [small.tile([P, 1], fp32, tag=f"r{i}", name=f"rs{i}") for i in range(BPC)]
        Qb = [None] * BPC

        def load_as(i):
            """Load K/V/Q for memory i, build bf16 [K | cV], A and S^T."""
            k_sb = kp.tile([P, R16, DK], fp32, tag="k", name=f"k{i}")
            nc.sync.dma_start(k_sb[:], keys[i].rearrange("(p r) k -> p r k", p=P))
            v_sb = vp.tile([P, R16, DV], fp32, tag="v", name=f"v{i}")
            nc.sync.dma_start(v_sb[:], values[i].rearrange("(p r) k -> p r k", p=P))
            q_sb = qp.tile([P, R16, DK], fp32, tag="q", name=f"q{i}")
            nc.scalar.dma_start(q_sb[:], queries[i].rearrange("(p r) k -> p r k", p=P))

            kvb = kvbp.tile([P, R16, 2 * P], bf16, tag="kvb", name=f"kvb{i}")
            # K cast on ScalarE (cheapest converter); Q cast on GpSimd
            nc.scalar.copy(out=kvb[:, :, 0:DK], in_=k_sb[:])
            Qb[i] = qbp.tile([P, R16, DK], bf16, tag=f"qb{i}", name=f"qb{i}")
            nc.gpsimd.tensor_copy(out=Qb[i][:], in_=q_sb[:])
            # V * c fp32 in place on DVE (fast), then cast to bf16 on ScalarE
            nc.vector.tensor_tensor(
                v_sb[:], v_sb[:],
                c_t[:, i, :, None].to_broadcast((P, R16, DV)),
                OP.mult,
            )
            nc.scalar.copy(out=kvb[:, :, DK : 2 * DK], in_=v_sb[:])

            ps = ps_as.tile([P, 2 * P], fp32, tag="as", name=f"ps_as{i}")
            for r in range(R16):
                nc.tensor.matmul(
                    ps[:], kvb[:, r, 0:DK], kvb[:, r, :],
                    start=(r == 0), stop=(r == R16 - 1),
                )
            nc.vector.tensor_tensor(A_sb[i][:], ps[:, 0:P], ident[:], OP.add)
            nc.vector.tensor_copy(out=ST_sb[i][:], in_=ps[:, P : 2 * P])
            nc.scalar.copy(out=A_bf[i][:], in_=A_sb[i][:])
            nc.vector.tensor_reduce(
                rs_sb[i][:], A_sb[i][:], AX.X, OP.add, apply_absolute_value=True
            )
            nc.vector.reciprocal(rs_sb[i][:], rs_sb[i][:])

        NGRP = 2
        GSZ = BPC // NGRP
        Xg = [None] * NGRP

        def x0(g):
            xw = xs.tile([P, GSZ * P], bf16, tag=f"Xb{g}", name=f"Xb{g}_0")
            for i in range(GSZ):
                nc.scalar.activation(
                    xw[:, i * P : (i + 1) * P], ident[:], AF.Copy,
                    scale=rs_sb[GSZ * g + i][:],
                )
            Xg[g] = xw

        def ns_stage(it):
            """One residual-form NS iteration for ALL groups, stage-interleaved
            so each group's serial DVE step hides behind the other group's
            matmuls in the TensorEngine FIFO."""
            bf_iter = it < ns_bf
            last_bf = it == ns_bf - 1
            Amat = A_bf if bf_iter else A_sb
            pas = []
            for g in range(NGRP):
                pa = ps_w.tile([P, GSZ * P], fp32, tag="w", name=f"pa{g}_{it}")
                for i in range(GSZ):
                    sl = slice(i * P, (i + 1) * P)
                    nc.tensor.matmul(pa[:, sl], Amat[GSZ * g + i][:], Xg[g][:, sl])
                pas.append(pa)
            egs = []
            for g in range(NGRP):
                eg = xs.tile(
                    [P, GSZ * P], bf16 if bf_iter else fp32,
                    tag=f"e{g}_{bf_iter}", name=f"e{g}_{it}",
                )
                nc.vector.scalar_tensor_tensor(
                    eg[:], pas[g][:], -1.0, ident4[:, 0 : GSZ * P], OP.mult, OP.add
                )
                egs.append(eg)
            pbs = []
            for g in range(NGRP):
                pb = ps_w.tile([P, GSZ * P], fp32, tag="w", name=f"pb{g}_{it}")
                for i in range(GSZ):
                    sl = slice(i * P, (i + 1) * P)
                    nc.tensor.matmul(pb[:, sl], Xg[g][:, sl], egs[g][:, sl])
                pbs.append(pb)
            out_fp32 = (not bf_iter) or last_bf
            for g in range(NGRP):
                xn = xs.tile(
                    [P, GSZ * P], fp32 if out_fp32 else bf16,
                    tag=f"Xf{g}" if out_fp32 else f"Xb{g}",
                    name=f"X{g}_{it + 1}",
                )
                nc.vector.tensor_tensor(xn[:], Xg[g][:], pbs[g][:], OP.add)
                Xg[g] = xn

        def phi(i):
            g, sl = i // GSZ, slice((i % GSZ) * P, (i % GSZ + 1) * P)
            ps_phi = ps_w.tile([P, P], fp32, tag="w", name=f"ps_phi{i}")
            nc.tensor.matmul(ps_phi[:], Xg[g][:, sl], ST_sb[i][:])
            nc.scalar.copy(out=Phi_bf[i][:], in_=ps_phi[:])

        qt_sb = [None] * BPC

        def qtrans(i):
            qt = qtp.tile([P, R16, P], bf16, tag="qt", name=f"qt{i}")
            for r4 in range(R16 // 4):
                ps_qt = ps_w.tile([P, 4 * P], bf16, tag="w", name=f"ps_qt{i}_{r4}")
                for j in range(4):
                    nc.tensor.transpose(
                        ps_qt[:, j * P : (j + 1) * P], Qb[i][:, 4 * r4 + j, :],
                        ident_bf[:],
                    )
                nc.vector.tensor_copy(
                    out=qt[:, 4 * r4 : 4 * r4 + 4, :], in_=ps_qt[:]
                )
            qt_sb[i] = qt

        def romms(i):
            o_sb = outp.tile([P, R16, DV], fp32, tag="o", name=f"o{i}")
            for r4 in range(R16 // 4):
                ps_o = ps_w.tile([P, 4 * P], fp32, tag="w", name=f"ps_o{i}_{r4}")
                for j in range(4):
                    nc.tensor.matmul(
                        ps_o[:, j * P : (j + 1) * P], qt_sb[i][:, 4 * r4 + j, :],
                        Phi_bf[i][:],
                    )
                nc.scalar.copy(
                    out=o_sb[:, 4 * r4 : 4 * r4 + 4, :], in_=ps_o[:]
                )
            nc.scalar.dma_start(out[i].rearrange("(p r) v -> p r v", p=P), o_sb[:])

        # ---- emission: loads, NS (groups alternating), phi, readout ----
        for i in range(BPC):
            load_as(i)
        for g in range(NGRP):
            x0(g)
        for it in range(NIT):
            ns_stage(it)
        for i in range(BPC):
            phi(i)
        for i in range(BPC):
            qtrans(i)
            romms(i)
        for pool in (ps_w, ps_as, outp, xs, small, qtp, qbp, qp, kvbp, vp,
                     kp, gam, const):
            pool.release()

    if not nc.is_finalized():
        nc.finalize()
    return nc


def kernel(**inputs) -> np.ndarray:
    keys = np.ascontiguousarray(inputs["keys"], dtype=np.float32)
    values = np.ascontiguousarray(inputs["values"], dtype=np.float32)
    gammas = np.ascontiguousarray(inputs["gammas"], dtype=np.float32)
    queries = np.ascontiguousarray(inputs["queries"], dtype=np.float32)

    from concourse.bass_utils import run_bass_kernel_spmd

    nc = build_nc()
    in_maps = []
    for m in range(NCORES):
        s = slice(m * BPC, (m + 1) * BPC)
        in_maps.append(
            {
                "keys": keys[s],
                "values": values[s],
                "gammas": gammas[s],
                "queries": queries[s],
            }
        )
    res = run_bass_kernel_spmd(nc, in_maps, core_ids=list(range(NCORES)))
    return np.concatenate([res.results[m]["out"] for m in range(NCORES)], axis=0)



# revision 3
# speedup vs baseline: 1.3350x; 1.3350x over previous
"""Mesa-layer memory kernel for Trainium2 (8 NeuronCores, data-parallel over B).

Math: the reference's T-step Sherman-Morrison / discounted-accumulation
recurrence has a closed form,
    R_final = (I + K^T K)^{-1}            (eps term is O(1e-6) relative)
    S_final^T = K^T diag(c) V,   c_t = prod_{s>t} gamma_s
so per memory b the output is
    out_b = Q_b @ (R_b @ S_b^T).
R is inverted with 5 Newton-Schulz iterations in residual form
    X <- X + X^T (I - A X)
run entirely in fp16 (1 cycle/row on the PE, 10 mantissa bits; validated
1.2e-3 max-rel vs fp64 closed form in numpy simulation, 16x under the
2e-2 gate). The output is stored to HBM in fp16 as well, halving the
output traffic; the host upcasts.

Layout trick: timestep t maps to (partition p, slot r) via t = 16 p + r,
making every DMA a fully contiguous 8 KB-per-partition transfer.

The suffix cumprod of gammas runs in log space: 16-step free-dim scans
plus one triangular matmul for the cross-partition prefix.

Emission is software-pipelined so the DMA stream never stalls: the A/S
contractions of memories 4-7 are interleaved into the serial dependency
gaps of group 0's Newton-Schulz iterations, and group 0's readout is
interleaved with group 1's iterations. K/V loads are issued ahead of Q
loads on the same queue so the recurrence-critical data arrives first.

Each core owns B/8 = 8 independent memories; no cross-core communication.
"""

import numpy as np

B, T, DK, DV, NQ = 64, 2048, 128, 128, 2048
NCORES = 8
BPC = B // NCORES          # memories per core
P = 128                    # partitions
R16 = T // P               # 16 row-slots per partition
GCLAMP = 1e-30             # gamma clamp before log (exact-0 gammas)
NS_IT = 5                  # Newton-Schulz iterations (all fp16)
NGRP = 2
GSZ = BPC // NGRP


def build_nc(ns_it=NS_IT):
    import concourse.mybir as mybir
    import concourse.tile as tile
    from concourse import bacc
    from concourse.masks import make_identity, make_upper_triangular

    fp32 = mybir.dt.float32
    fp16 = mybir.dt.float16
    AF = mybir.ActivationFunctionType
    OP = mybir.AluOpType
    AX = mybir.AxisListType

    nc = bacc.Bacc(trn_type="TRN2", target_bir_lowering=False, debug=False)
    keys = nc.dram_tensor("keys", [BPC, T, DK], fp32, kind="ExternalInput").ap()
    values = nc.dram_tensor("values", [BPC, T, DV], fp32, kind="ExternalInput").ap()
    gammas = nc.dram_tensor("gammas", [BPC, T], fp32, kind="ExternalInput").ap()
    queries = nc.dram_tensor("queries", [BPC, NQ, DK], fp32, kind="ExternalInput").ap()
    out = nc.dram_tensor("out", [BPC, NQ, DV], fp16, kind="ExternalOutput").ap()

    with tile.TileContext(nc) as tc:
        const = tc.alloc_tile_pool(name="const", bufs=1)
        gam = tc.alloc_tile_pool(name="gam", bufs=1)
        kp = tc.alloc_tile_pool(name="kp", bufs=3)
        vp = tc.alloc_tile_pool(name="vp", bufs=3)
        kvbp = tc.alloc_tile_pool(name="kvbp", bufs=2)
        qp = tc.alloc_tile_pool(name="qp", bufs=3)
        qhp = tc.alloc_tile_pool(name="qhp", bufs=2)
        qtp = tc.alloc_tile_pool(name="qtp", bufs=2)
        small = tc.alloc_tile_pool(name="small", bufs=1)
        xs = tc.alloc_tile_pool(name="xs", bufs=2)
        outp = tc.alloc_tile_pool(name="outp", bufs=2)
        ps_sm = tc.alloc_tile_pool(name="ps_sm", bufs=2, space="PSUM")
        ps_ns = tc.alloc_tile_pool(name="ps_ns", bufs=3, space="PSUM")
        ps_rd = tc.alloc_tile_pool(name="ps_rd", bufs=3, space="PSUM")

        ident = const.tile([P, P], fp32)
        make_identity(nc, ident)
        ident_h = const.tile([P, P], fp16)
        make_identity(nc, ident_h)
        ident4 = const.tile([P, GSZ * P], fp32)
        for i in range(GSZ):
            make_identity(nc, ident4[:, i * P : (i + 1) * P])
        utri = const.tile([P, P], fp32)
        make_upper_triangular(nc, utri, val=1.0, diag=False)
        ones2 = const.tile([P, P], fp32)
        nc.gpsimd.memset(ones2[:], 1.0)

        # ---- DMA issue: gammas first (tiny), then all K/V, then Q ----
        g16 = gam.tile([P, BPC, R16], fp32)
        nc.sync.dma_start(g16[:], gammas.rearrange("i (p r) -> p i r", r=R16))
        k_sb = [None] * BPC
        v_sb = [None] * BPC
        q_sb = [None] * BPC
        for i in range(BPC):
            k_sb[i] = kp.tile([P, R16, DK], fp32, tag="k", name=f"k{i}")
            nc.sync.dma_start(k_sb[i][:], keys[i].rearrange("(p r) k -> p r k", p=P))
            v_sb[i] = vp.tile([P, R16, DV], fp32, tag="v", name=f"v{i}")
            nc.sync.dma_start(v_sb[i][:], values[i].rearrange("(p r) k -> p r k", p=P))
        for i in range(BPC):
            q_sb[i] = qp.tile([P, R16, DK], fp32, tag="q", name=f"q{i}")
            nc.sync.dma_start(q_sb[i][:], queries[i].rearrange("(p r) k -> p r k", p=P))

        # ---- phase 0: suffix cumprod of gammas (log space) ----
        g16f = g16.rearrange("p i r -> p (i r)")
        nc.vector.tensor_scalar_max(g16f, g16f, GCLAMP)
        nc.scalar.activation(g16f, g16f, AF.Ln)
        incl = gam.tile([P, BPC, R16], fp32)
        zz = gam.tile([P, R16], fp32)
        nc.vector.memset(zz[:], 0.0)
        # joiner: make DVE observe the ACT (Ln) dependency before the scans
        joiner = gam.tile([P, 1], fp32)
        nc.vector.tensor_copy(out=joiner[:], in_=g16[:, 0, 0:1])
        for i in range(BPC):
            nc.vector.tensor_tensor_scan(
                incl[:, i, :], g16[:, i, :], zz[:], 0.0, OP.add, OP.add
            )
        ptot = gam.tile([P, BPC], fp32)
        nc.vector.tensor_copy(out=ptot[:], in_=incl[:, :, R16 - 1])
        ps_pre = ps_sm.tile([P, 2 * BPC], fp32, tag="sm", name="ps_pre")
        nc.tensor.matmul(ps_pre[:, 0:BPC], utri[:], ptot[:])
        nc.tensor.matmul(ps_pre[:, BPC : 2 * BPC], ones2[:], ptot[:])
        pre_sb = gam.tile([P, 2 * BPC], fp32)
        nc.vector.tensor_copy(out=pre_sb[:], in_=ps_pre[:])
        bias2 = gam.tile([P, BPC], fp32)
        nc.vector.tensor_tensor(
            bias2[:], pre_sb[:, BPC : 2 * BPC], pre_sb[:, 0:BPC], OP.subtract
        )
        c_t = gam.tile([P, BPC, R16], fp32)
        for i in range(BPC):
            nc.scalar.activation(
                c_t[:, i, :], incl[:, i, :], AF.Exp,
                bias=bias2[:, i : i + 1], scale=-1.0,
            )

        # ---- per-memory state tiles ----
        A_lp = [small.tile([P, P], fp16, tag=f"A{i}", name=f"A{i}") for i in range(BPC)]
        ST_lp = [small.tile([P, P], fp16, tag=f"S{i}", name=f"S{i}") for i in range(BPC)]
        Phi_lp = [small.tile([P, P], fp16, tag=f"P{i}", name=f"Phi{i}") for i in range(BPC)]
        rs_sb = [small.tile([P, 1], fp32, tag=f"r{i}", name=f"rs{i}") for i in range(BPC)]
        qt_sb = [None] * BPC
        Xg = [None] * NGRP

        def prep(i):
            """kv build (fp16) + A/S contraction + A_lp/ST/rs for memory i."""
            kvb = kvbp.tile([P, R16, 2 * P], fp16, tag="kvb", name=f"kvb{i}")
            nc.scalar.copy(out=kvb[:, :, 0:DK], in_=k_sb[i][:])
            nc.vector.tensor_tensor(
                kvb[:, :, DK : 2 * DK], v_sb[i][:],
                c_t[:, i, :, None].to_broadcast((P, R16, DV)),
                OP.mult,
            )
            ps = ps_sm.tile([P, 2 * P], fp32, tag="sm", name=f"ps_as{i}")
            for r in range(R16):
                nc.tensor.matmul(
                    ps[:], kvb[:, r, 0:DK], kvb[:, r, :],
                    start=(r == 0), stop=(r == R16 - 1),
                )
            nc.vector.tensor_tensor(A_lp[i][:], ps[:, 0:P], ident[:], OP.add)
            nc.scalar.copy(out=ST_lp[i][:], in_=ps[:, P : 2 * P])
            nc.vector.tensor_reduce(
                rs_sb[i][:], A_lp[i][:], AX.X, OP.add, apply_absolute_value=True
            )
            nc.vector.reciprocal(rs_sb[i][:], rs_sb[i][:])

        def x0(g):
            xw = xs.tile([P, GSZ * P], fp16, tag=f"X{g}", name=f"X{g}_0")
            for i in range(GSZ):
                nc.scalar.activation(
                    xw[:, i * P : (i + 1) * P], ident[:], AF.Copy,
                    scale=rs_sb[GSZ * g + i][:],
                )
            Xg[g] = xw

        pa_ps = [None] * NGRP
        eg_sb = [None] * NGRP

        def ns_a(g, it):
            """pa = A @ X (4 matmuls) + eg = I - pa (DVE)."""
            pa = ps_ns.tile([P, GSZ * P], fp32, tag="ns", name=f"pa{g}_{it}")
            for i in range(GSZ):
                sl = slice(i * P, (i + 1) * P)
                nc.tensor.matmul(pa[:, sl], A_lp[GSZ * g + i][:], Xg[g][:, sl])
            eg = xs.tile([P, GSZ * P], fp16, tag=f"e{g}", name=f"e{g}_{it}")
            nc.vector.scalar_tensor_tensor(
                eg[:], pa[:], -1.0, ident4[:], OP.mult, OP.add
            )
            pa_ps[g] = pa
            eg_sb[g] = eg

        def ns_b(g, it):
            """pb = X @ eg (4 matmuls) + X' = X + pb (DVE)."""
            pb = ps_ns.tile([P, GSZ * P], fp32, tag="ns", name=f"pb{g}_{it}")
            for i in range(GSZ):
                sl = slice(i * P, (i + 1) * P)
                nc.tensor.matmul(pb[:, sl], Xg[g][:, sl], eg_sb[g][:, sl])
            xn = xs.tile([P, GSZ * P], fp16, tag=f"X{g}", name=f"X{g}_{it + 1}")
            nc.vector.tensor_tensor(xn[:], Xg[g][:], pb[:], OP.add)
            Xg[g] = xn

        def phi(i):
            g, sl = i // GSZ, slice((i % GSZ) * P, (i % GSZ + 1) * P)
            ps_phi = ps_sm.tile([P, P], fp32, tag="sm", name=f"ps_phi{i}")
            nc.tensor.matmul(ps_phi[:], Xg[g][:, sl], ST_lp[i][:])
            nc.scalar.copy(out=Phi_lp[i][:], in_=ps_phi[:])

        def qtr(i):
            """Cast Q to fp16 and transpose all 16 slots on the PE."""
            qh = qhp.tile([P, R16, DK], fp16, tag="qh", name=f"qh{i}")
            nc.scalar.copy(out=qh[:], in_=q_sb[i][:])
            qt = qtp.tile([P, R16, P], fp16, tag="qt", name=f"qt{i}")
            for r4 in range(R16 // 4):
                ps_qt = ps_rd.tile([P, 4 * P], fp16, tag="rd", name=f"ps_qt{i}_{r4}")
                for j in range(4):
                    nc.tensor.transpose(
                        ps_qt[:, j * P : (j + 1) * P], qh[:, 4 * r4 + j, :],
                        ident_h[:],
                    )
                nc.vector.tensor_copy(
                    out=qt[:, 4 * r4 : 4 * r4 + 4, :], in_=ps_qt[:]
                )
            qt_sb[i] = qt

        def rom(i):
            o_sb = outp.tile([P, R16, DV], fp16, tag="o", name=f"o{i}")
            for r4 in range(R16 // 4):
                ps_o = ps_rd.tile([P, 4 * P], fp32, tag="rd", name=f"ps_o{i}_{r4}")
                for j in range(4):
                    nc.tensor.matmul(
                        ps_o[:, j * P : (j + 1) * P], qt_sb[i][:, 4 * r4 + j, :],
                        Phi_lp[i][:],
                    )
                nc.scalar.copy(
                    out=o_sb[:, 4 * r4 : 4 * r4 + 4, :], in_=ps_o[:]
                )
            nc.scalar.dma_start(out[i].rearrange("(p r) v -> p r v", p=P), o_sb[:])

        # ---- pipelined emission ----
        for i in range(4):
            prep(i)
        x0(0)
        ns_a(0, 0)
        prep(4)
        ns_b(0, 0)
        ns_a(0, 1)
        prep(5)
        ns_b(0, 1)
        ns_a(0, 2)
        prep(6)
        ns_b(0, 2)
        ns_a(0, 3)
        prep(7)
        x0(1)
        ns_b(0, 3)
        ns_a(0, 4)
        ns_a(1, 0)
        ns_b(0, 4)
        ns_b(1, 0)
        for i in range(4):
            phi(i)
        ns_a(1, 1)
        qtr(0)
        ns_b(1, 1)
        rom(0)
        ns_a(1, 2)
        qtr(1)
        ns_b(1, 2)
        rom(1)
        ns_a(1, 3)
        qtr(2)
        ns_b(1, 3)
        rom(2)
        ns_a(1, 4)
        qtr(3)
        ns_b(1, 4)
        for i in range(4, 8):
            phi(i)
        rom(3)
        for i in range(4, 8):
            qtr(i)
            rom(i)
        for pool in (ps_rd, ps_ns, ps_sm, outp, xs, small, qtp, qhp, qp,
                     kvbp, vp, kp, gam, const):
            pool.release()

    if not nc.is_finalized():
        nc.finalize()
    return nc


def kernel(**inputs) -> np.ndarray:
    keys = np.ascontiguousarray(inputs["keys"], dtype=np.float32)
    values = np.ascontiguousarray(inputs["values"], dtype=np.float32)
    gammas = np.ascontiguousarray(inputs["gammas"], dtype=np.float32)
    queries = np.ascontiguousarray(inputs["queries"], dtype=np.float32)

    from concourse.bass_utils import run_bass_kernel_spmd

    nc = build_nc()
    in_maps = []
    for m in range(NCORES):
        s = slice(m * BPC, (m + 1) * BPC)
        in_maps.append(
            {
                "keys": keys[s],
                "values": values[s],
                "gammas": gammas[s],
                "queries": queries[s],
            }
        )
    res = run_bass_kernel_spmd(nc, in_maps, core_ids=list(range(NCORES)))
    return np.concatenate(
        [res.results[m]["out"] for m in range(NCORES)], axis=0
    ).astype(np.float32)


# revision 6
# speedup vs baseline: 1.4512x; 1.0870x over previous
"""Mesa-layer memory kernel for Trainium2 (8 NeuronCores, data-parallel over B).

Math: the reference's T-step Sherman-Morrison / discounted-accumulation
recurrence has a closed form,
    R_final = (I + K^T K)^{-1}            (eps term is O(1e-6) relative)
    S_final^T = K^T diag(c) V,   c_t = prod_{s>t} gamma_s
so per memory b the output is
    out_b = Q_b @ (R_b @ S_b^T).
R is inverted with 5 Newton-Schulz iterations in residual form
    X <- X + X^T (I - A X)
run entirely in fp16 (1 cycle/row on the PE, 10 mantissa bits; validated
1.2e-3 max-rel vs fp64 closed form in numpy simulation, 16x under the
2e-2 gate). The output is stored to HBM in fp16 as well, halving the
output traffic; the host upcasts.

Layout trick: timestep t maps to (partition p, slot r) via t = 16 p + r,
making every DMA a fully contiguous 8 KB-per-partition transfer.

The suffix cumprod of gammas runs in log space: 16-step free-dim scans
plus one triangular matmul for the cross-partition prefix.

Emission is software-pipelined so the DMA stream never stalls: the A/S
contractions of memories 4-7 are interleaved into the serial dependency
gaps of group 0's Newton-Schulz iterations, and group 0's readout is
interleaved with group 1's iterations. K/V loads are issued ahead of Q
loads on the same queue so the recurrence-critical data arrives first.

Each core owns B/8 = 8 independent memories; no cross-core communication.
"""

import numpy as np

B, T, DK, DV, NQ = 64, 2048, 128, 128, 2048
NCORES = 8
BPC = B // NCORES          # memories per core
P = 128                    # partitions
R16 = T // P               # 16 row-slots per partition
GCLAMP = 1e-30             # gamma clamp before log (exact-0 gammas)
NS_IT = 5                  # Newton-Schulz iterations (all fp16)
NGRP = 2
GSZ = BPC // NGRP


def build_nc(ns_it=NS_IT):
    import concourse.mybir as mybir
    import concourse.tile as tile
    from concourse import bacc
    from concourse.masks import make_identity, make_upper_triangular

    fp32 = mybir.dt.float32
    fp16 = mybir.dt.float16
    AF = mybir.ActivationFunctionType
    OP = mybir.AluOpType
    AX = mybir.AxisListType

    nc = bacc.Bacc(trn_type="TRN2", target_bir_lowering=False, debug=False)
    keys = nc.dram_tensor("keys", [BPC, T, DK], fp32, kind="ExternalInput").ap()
    values = nc.dram_tensor("values", [BPC, T, DV], fp32, kind="ExternalInput").ap()
    gammas = nc.dram_tensor("gammas", [BPC, T], fp32, kind="ExternalInput").ap()
    queries = nc.dram_tensor("queries", [BPC, NQ, DK], fp32, kind="ExternalInput").ap()
    out = nc.dram_tensor("out", [BPC, NQ, DV], fp16, kind="ExternalOutput").ap()

    with tile.TileContext(nc) as tc:
        const = tc.alloc_tile_pool(name="const", bufs=1)
        gam = tc.alloc_tile_pool(name="gam", bufs=1)
        kp = tc.alloc_tile_pool(name="kp", bufs=3)
        vp = tc.alloc_tile_pool(name="vp", bufs=3)
        kvbp = tc.alloc_tile_pool(name="kvbp", bufs=2)
        qp = tc.alloc_tile_pool(name="qp", bufs=BPC)
        qhp = tc.alloc_tile_pool(name="qhp", bufs=2)
        qtp = tc.alloc_tile_pool(name="qtp", bufs=2)
        small = tc.alloc_tile_pool(name="small", bufs=1)
        xs = tc.alloc_tile_pool(name="xs", bufs=2)
        outp = tc.alloc_tile_pool(name="outp", bufs=2)
        ps_sm = tc.alloc_tile_pool(name="ps_sm", bufs=2, space="PSUM")
        ps_ns = tc.alloc_tile_pool(name="ps_ns", bufs=3, space="PSUM")
        ps_rd = tc.alloc_tile_pool(name="ps_rd", bufs=3, space="PSUM")

        ident = const.tile([P, P], fp32)
        make_identity(nc, ident)
        ident_h = const.tile([P, P], fp16)
        make_identity(nc, ident_h)
        ident4 = const.tile([P, GSZ * P], fp32)
        for i in range(GSZ):
            make_identity(nc, ident4[:, i * P : (i + 1) * P])
        utri = const.tile([P, P], fp32)
        make_upper_triangular(nc, utri, val=1.0, diag=False)
        ones2 = const.tile([P, P], fp32)
        nc.gpsimd.memset(ones2[:], 1.0)

        # ---- DMA issue: gammas first (tiny), then all K/V, then Q ----
        g16 = gam.tile([P, BPC, R16], fp32)
        nc.sync.dma_start(g16[:], gammas.rearrange("i (p r) -> p i r", r=R16))
        k_sb = [None] * BPC
        v_sb = [None] * BPC
        q_sb = [None] * BPC
        for i in range(BPC):
            k_sb[i] = kp.tile([P, R16, DK], fp32, tag="k", name=f"k{i}")
            nc.sync.dma_start(k_sb[i][:], keys[i].rearrange("(p r) k -> p r k", p=P))
            v_sb[i] = vp.tile([P, R16, DV], fp32, tag="v", name=f"v{i}")
            nc.sync.dma_start(v_sb[i][:], values[i].rearrange("(p r) k -> p r k", p=P))
        for i in range(BPC):
            q_sb[i] = qp.tile([P, R16, DK], fp32, tag="q", name=f"q{i}")
            nc.sync.dma_start(q_sb[i][:], queries[i].rearrange("(p r) k -> p r k", p=P))

        # ---- phase 0: suffix cumprod of gammas (log space) ----
        g16f = g16.rearrange("p i r -> p (i r)")
        nc.vector.tensor_scalar_max(g16f, g16f, GCLAMP)
        nc.scalar.activation(g16f, g16f, AF.Ln)
        incl = gam.tile([P, BPC, R16], fp32)
        zz = gam.tile([P, R16], fp32)
        nc.vector.memset(zz[:], 0.0)
        # joiner: make DVE observe the ACT (Ln) dependency before the scans
        joiner = gam.tile([P, 1], fp32)
        nc.vector.tensor_copy(out=joiner[:], in_=g16[:, 0, 0:1])
        for i in range(BPC):
            nc.vector.tensor_tensor_scan(
                incl[:, i, :], g16[:, i, :], zz[:], 0.0, OP.add, OP.add
            )
        ptot = gam.tile([P, BPC], fp32)
        nc.vector.tensor_copy(out=ptot[:], in_=incl[:, :, R16 - 1])
        ps_pre = ps_sm.tile([P, 2 * BPC], fp32, tag="sm", name="ps_pre")
        nc.tensor.matmul(ps_pre[:, 0:BPC], utri[:], ptot[:])
        nc.tensor.matmul(ps_pre[:, BPC : 2 * BPC], ones2[:], ptot[:])
        pre_sb = gam.tile([P, 2 * BPC], fp32)
        nc.vector.tensor_copy(out=pre_sb[:], in_=ps_pre[:])
        bias2 = gam.tile([P, BPC], fp32)
        nc.vector.tensor_tensor(
            bias2[:], pre_sb[:, BPC : 2 * BPC], pre_sb[:, 0:BPC], OP.subtract
        )
        c_t = gam.tile([P, BPC, R16], fp32)
        for i in range(BPC):
            nc.scalar.activation(
                c_t[:, i, :], incl[:, i, :], AF.Exp,
                bias=bias2[:, i : i + 1], scale=-1.0,
            )

        # ---- per-memory state tiles ----
        A_lp = [small.tile([P, P], fp16, tag=f"A{i}", name=f"A{i}") for i in range(BPC)]
        ST_lp = [small.tile([P, P], fp16, tag=f"S{i}", name=f"S{i}") for i in range(BPC)]
        Phi_lp = [small.tile([P, P], fp16, tag=f"P{i}", name=f"Phi{i}") for i in range(BPC)]
        rs_sb = [small.tile([P, 1], fp32, tag=f"r{i}", name=f"rs{i}") for i in range(BPC)]
        qt_sb = [None] * BPC
        Xg = [None] * NGRP

        def prep(i):
            """kv build (fp16) + A/S contraction + A_lp/ST/rs for memory i."""
            kvb = kvbp.tile([P, R16, 2 * P], fp16, tag="kvb", name=f"kvb{i}")
            nc.scalar.copy(out=kvb[:, :, 0:DK], in_=k_sb[i][:])
            nc.vector.tensor_tensor(
                kvb[:, :, DK : 2 * DK], v_sb[i][:],
                c_t[:, i, :, None].to_broadcast((P, R16, DV)),
                OP.mult,
            )
            ps = ps_sm.tile([P, 2 * P], fp32, tag="sm", name=f"ps_as{i}")
            for r in range(R16):
                nc.tensor.matmul(
                    ps[:], kvb[:, r, 0:DK], kvb[:, r, :],
                    start=(r == 0), stop=(r == R16 - 1),
                )
            nc.vector.tensor_tensor(A_lp[i][:], ps[:, 0:P], ident[:], OP.add)
            nc.scalar.copy(out=ST_lp[i][:], in_=ps[:, P : 2 * P])
            nc.vector.tensor_reduce(
                rs_sb[i][:], A_lp[i][:], AX.X, OP.add, apply_absolute_value=True
            )
            nc.vector.reciprocal(rs_sb[i][:], rs_sb[i][:])

        def x0(g):
            xw = xs.tile([P, GSZ * P], fp16, tag=f"X{g}", name=f"X{g}_0")
            for i in range(GSZ):
                nc.scalar.activation(
                    xw[:, i * P : (i + 1) * P], ident[:], AF.Copy,
                    scale=rs_sb[GSZ * g + i][:],
                )
            Xg[g] = xw

        pa_ps = [None] * NGRP
        eg_sb = [None] * NGRP

        def ns_a(g, it):
            """pa = A @ X (4 matmuls) + eg = I - pa (DVE)."""
            pa = ps_ns.tile([P, GSZ * P], fp32, tag="ns", name=f"pa{g}_{it}")
            for i in range(GSZ):
                sl = slice(i * P, (i + 1) * P)
                nc.tensor.matmul(pa[:, sl], A_lp[GSZ * g + i][:], Xg[g][:, sl])
            eg = xs.tile([P, GSZ * P], fp16, tag=f"e{g}", name=f"e{g}_{it}")
            nc.vector.scalar_tensor_tensor(
                eg[:], pa[:], -1.0, ident4[:], OP.mult, OP.add
            )
            pa_ps[g] = pa
            eg_sb[g] = eg

        def ns_b(g, it):
            """pb = X @ eg (4 matmuls) + X' = X + pb (DVE)."""
            pb = ps_ns.tile([P, GSZ * P], fp32, tag="ns", name=f"pb{g}_{it}")
            for i in range(GSZ):
                sl = slice(i * P, (i + 1) * P)
                nc.tensor.matmul(pb[:, sl], Xg[g][:, sl], eg_sb[g][:, sl])
            xn = xs.tile([P, GSZ * P], fp16, tag=f"X{g}", name=f"X{g}_{it + 1}")
            nc.vector.tensor_tensor(xn[:], Xg[g][:], pb[:], OP.add)
            Xg[g] = xn

        def phi(i):
            g, sl = i // GSZ, slice((i % GSZ) * P, (i % GSZ + 1) * P)
            ps_phi = ps_sm.tile([P, P], fp32, tag="sm", name=f"ps_phi{i}")
            nc.tensor.matmul(ps_phi[:], Xg[g][:, sl], ST_lp[i][:])
            nc.scalar.copy(out=Phi_lp[i][:], in_=ps_phi[:])

        def qtr(i):
            """Cast Q to fp16 and transpose all 16 slots on the PE."""
            qh = qhp.tile([P, R16, DK], fp16, tag="qh", name=f"qh{i}")
            nc.scalar.copy(out=qh[:], in_=q_sb[i][:])
            qt = qtp.tile([P, R16, P], fp16, tag="qt", name=f"qt{i}")
            for r4 in range(R16 // 4):
                ps_qt = ps_rd.tile([P, 4 * P], fp16, tag="rd", name=f"ps_qt{i}_{r4}")
                for j in range(4):
                    nc.tensor.transpose(
                        ps_qt[:, j * P : (j + 1) * P], qh[:, 4 * r4 + j, :],
                        ident_h[:],
                    )
                nc.vector.tensor_copy(
                    out=qt[:, 4 * r4 : 4 * r4 + 4, :], in_=ps_qt[:]
                )
            qt_sb[i] = qt

        def rom(i):
            o_sb = outp.tile([P, R16, DV], fp16, tag="o", name=f"o{i}")
            for r4 in range(R16 // 4):
                ps_o = ps_rd.tile([P, 4 * P], fp32, tag="rd", name=f"ps_o{i}_{r4}")
                for j in range(4):
                    nc.tensor.matmul(
                        ps_o[:, j * P : (j + 1) * P], qt_sb[i][:, 4 * r4 + j, :],
                        Phi_lp[i][:],
                    )
                # split the PSUM->SBUF cast copies between Scalar and DVE
                eng = nc.scalar if r4 % 2 == 0 else nc.vector
                if eng is nc.scalar:
                    eng.copy(out=o_sb[:, 4 * r4 : 4 * r4 + 4, :], in_=ps_o[:])
                else:
                    eng.tensor_copy(out=o_sb[:, 4 * r4 : 4 * r4 + 4, :], in_=ps_o[:])
            nc.gpsimd.dma_start(out[i].rearrange("(p r) v -> p r v", p=P), o_sb[:])

        # ---- pipelined emission ----
        for i in range(4):
            prep(i)
        x0(0)
        ns_a(0, 0)
        prep(4)
        ns_b(0, 0)
        ns_a(0, 1)
        prep(5)
        ns_b(0, 1)
        ns_a(0, 2)
        prep(6)
        ns_b(0, 2)
        ns_a(0, 3)
        prep(7)
        x0(1)
        ns_b(0, 3)
        ns_a(0, 4)
        ns_a(1, 0)
        ns_b(0, 4)
        ns_b(1, 0)
        for i in range(4):
            phi(i)
        ns_a(1, 1)
        qtr(0)
        ns_b(1, 1)
        rom(0)
        ns_a(1, 2)
        qtr(1)
        ns_b(1, 2)
        rom(1)
        ns_a(1, 3)
        qtr(2)
        ns_b(1, 3)
        rom(2)
        ns_a(1, 4)
        qtr(3)
        ns_b(1, 4)
        rom(3)
        for i in range(4, 8):
            phi(i)
        for i in range(4, 8):
            qtr(i)
            rom(i)
        for pool in (ps_rd, ps_ns, ps_sm, outp, xs, small, qtp, qhp, qp,
                     kvbp, vp, kp, gam, const):
            pool.release()

    if not nc.is_finalized():
        nc.finalize()
    return nc


def kernel(**inputs) -> np.ndarray:
    keys = np.ascontiguousarray(inputs["keys"], dtype=np.float32)
    values = np.ascontiguousarray(inputs["values"], dtype=np.float32)
    gammas = np.ascontiguousarray(inputs["gammas"], dtype=np.float32)
    queries = np.ascontiguousarray(inputs["queries"], dtype=np.float32)

    from concourse.bass_utils import run_bass_kernel_spmd

    nc = build_nc()
    in_maps = []
    for m in range(NCORES):
        s = slice(m * BPC, (m + 1) * BPC)
        in_maps.append(
            {
                "keys": keys[s],
                "values": values[s],
                "gammas": gammas[s],
                "queries": queries[s],
            }
        )
    res = run_bass_kernel_spmd(nc, in_maps, core_ids=list(range(NCORES)))
    return np.concatenate(
        [res.results[m]["out"] for m in range(NCORES)], axis=0
    ).astype(np.float32)
